# revision 2
# baseline (speedup 1.0000x reference)
"""Depthwise causal conv1d (K=4) Trainium2 kernel.

Problem: x (B=8, S=4096, F=2048) f32, conv_state (B, F, 3), weight (F, 1, 4),
bias (F,).  out[b, s, f] = bias[f] + sum_d weight[f,0,d] * xpad[b, s+d, f]
where xpad = concat(conv_state[b].T, x[b]) along time.  Also returns
new_conv_state = xpad[:, -3:, :].T (computed on host -- it is a pure
input slice).

Sharding: batch across the 8 NeuronCores (one batch item per core), weights
replicated.  No cross-device communication.

Per-core layout: S on partitions, F on the free dim (natural DMA: each
partition line is a contiguous 8KB row of x).  Tiles of 124 output rows are
built from 127 input rows (3-row halo re-read).  Per tile:
  1. y_d = x_tile * wrep_d on the Vector engine only (tensor_tensor mult,
     f32 -> float32r out).  GpSimd is NOT used: it shares an SBUF read port
     with DVE and tensor_tensor on both engines concurrently is a net loss.
  2. PE accumulates out = sum_d Shift_d @ y_d in PSUM via float32r matmuls
     (0/1 shift weights are exact; data pays f32r rounding ~1e-4 rel).
     Bias is folded in: y_0 row 127 holds the bias row (8KB SBUF->SBUF DMA)
     and Shift_0 row 127 is all-ones.
  3. ACT (ScalarE) evacuates PSUM -> SBUF, HWDGE DMA stores.
"""

import numpy as np

B, S, F, K = 8, 4096, 2048, 4
P = 128
TILE_OUT = 124  # output rows per tile; 127 input rows + bias row at 127
N_TILES = (S + TILE_OUT - 1) // TILE_OUT  # 34 (last tile: 4 rows)
CHUNK = 512  # PSUM bank = 512 fp32

_CACHE = {}
LAST_RESULTS = None
TRACE = False


def _build_bass():
    import concourse.tile as tile
    import concourse.bacc as bacc
    from concourse import mybir

    f32 = mybir.dt.float32
    f32r = mybir.dt.float32r

    nc = bacc.Bacc("TRN2", target_bir_lowering=False, debug=False)

    x_dram = nc.dram_tensor("x", (S, F), f32, kind="ExternalInput")
    state_dram = nc.dram_tensor("state", (K - 1, F), f32, kind="ExternalInput")
    wrep_dram = nc.dram_tensor("wrep", (K, P, F), f32, kind="ExternalInput")
    wshift_dram = nc.dram_tensor("wshift", (K, P, P), f32, kind="ExternalInput")
    biasrow_dram = nc.dram_tensor("biasrow", (1, F), f32, kind="ExternalInput")
    ones_dram = nc.dram_tensor("ones", (1, P), f32, kind="ExternalInput")
    out_dram = nc.dram_tensor("out", (S, F), f32, kind="ExternalOutput")

    NCH = F // CHUNK

    with tile.TileContext(nc) as tc:
        with (
            tc.tile_pool(name="consts", bufs=1) as consts,
            tc.tile_pool(name="xp", bufs=3) as xp,
            tc.tile_pool(name="yp", bufs=2) as yp,
            tc.tile_pool(name="op", bufs=3) as op,
            tc.tile_pool(name="psum", bufs=2, space="PSUM") as pp,
        ):
            wrep = []
            wshift = []
            for d in range(K):
                wr = consts.tile([P, F], f32, tag=f"wrep{d}")
                nc.sync.dma_start(wr[:], wrep_dram.ap()[d])
                wrep.append(wr)
                wsf = consts.tile([P, P], f32, tag=f"wshiftf{d}")
                nc.sync.dma_start(wsf[:], wshift_dram.ap()[d])
                ws = consts.tile([P, P], f32r, tag=f"wshift{d}")
                nc.vector.tensor_copy(ws[:], wsf[:])
                wshift.append(ws)
            biasrow_f = consts.tile([1, F], f32)
            nc.sync.dma_start(biasrow_f[:], biasrow_dram.ap())
            biasrow = consts.tile([1, F], f32r, tag="biasrow_r")
            nc.vector.tensor_copy(biasrow[:], biasrow_f[:])
            ones_f = consts.tile([1, P], f32)
            nc.sync.dma_start(ones_f[:], ones_dram.ap())
            ones = consts.tile([1, P], f32r, tag="ones_r")
            nc.vector.tensor_copy(ones[:], ones_f[:])

            for j in range(N_TILES):
                r0 = TILE_OUT * j
                n_out = min(TILE_OUT, S - r0)
                n_in = n_out + (K - 1)
                full = n_out == TILE_OUT

                xt = xp.tile([P, F], f32)
                if j == 0:
                    nc.sync.dma_start(xt[0 : K - 1, :], state_dram.ap())
                    nc.sync.dma_start(xt[K - 1 : n_in, :], x_dram.ap()[0:n_out, :])
                else:
                    nc.sync.dma_start(
                        xt[0:n_in, :], x_dram.ap()[r0 - (K - 1) : r0 + n_out, :]
                    )

                ys = []
                for d in range(K):
                    y = yp.tile([P, F], f32r, tag=f"y{d}")
                    nc.vector.tensor_mul(
                        y[0:n_in, :], xt[0:n_in, :], wrep[d][0:n_in, :]
                    )
                    ys.append(y)
                if full:
                    # bias row rides y_0 partition 127 (read by W_0's ones row)
                    nc.sync.dma_start(ys[0][127:128, :], biasrow[:])

                acc = pp.tile([P, F], f32)
                if full:
                    # d-outer order: weight reused across the 4 chunks
                    for d in range(K):
                        rows = P if d == 0 else n_in
                        for c in range(NCH):
                            sl = slice(CHUNK * c, CHUNK * (c + 1))
                            nc.tensor.matmul(
                                acc[:, sl],
                                wshift[d][0:rows, :],
                                ys[d][0:rows, sl],
                                start=(d == 0),
                                stop=(d == K - 1),
                            )
                else:
                    # short last tile: plain shifts + explicit bias matmul
                    for d in range(K):
                        for c in range(NCH):
                            sl = slice(CHUNK * c, CHUNK * (c + 1))
                            nc.tensor.matmul(
                                acc[:, sl],
                                wshift[d][0:n_in, :],
                                ys[d][0:n_in, sl],
                                start=(d == 0),
                                stop=False,
                            )
                    for c in range(NCH):
                        sl = slice(CHUNK * c, CHUNK * (c + 1))
                        nc.tensor.matmul(
                            acc[:, sl], ones[:], biasrow[:, sl],
                            start=False, stop=True,
                        )

                ot = op.tile([TILE_OUT, F], f32)
                nc.scalar.copy(ot[0:n_out, :], acc[0:n_out, :])
                nc.sync.dma_start(out_dram.ap()[r0 : r0 + n_out, :], ot[0:n_out, :])

    nc.compile()
    return nc


def _consts_np(weight, bias):
    w = weight[:, 0, :].astype(np.float32)  # (F, K)
    wrep = np.ascontiguousarray(
        np.broadcast_to(w.T[:, None, :], (K, P, F)), dtype=np.float32
    )
    wshift = np.zeros((K, P, P), dtype=np.float32)
    for d in range(K):
        for m in range(P - d):
            wshift[d, m + d, m] = 1.0
    wshift[0, 127, :] = 1.0  # bias row: out[:, m] += y0[127] = bias
    biasrow = np.ascontiguousarray(bias[None, :], dtype=np.float32)
    ones = np.ones((1, P), dtype=np.float32)
    return wrep, wshift, biasrow, ones


def kernel(x, conv_state, weight, bias):
    global LAST_RESULTS
    from concourse.bass_utils import run_bass_kernel_spmd

    x = np.asarray(x, dtype=np.float32)
    conv_state = np.asarray(conv_state, dtype=np.float32)
    weight = np.asarray(weight, dtype=np.float32)
    bias = np.asarray(bias, dtype=np.float32)

    if "nc" not in _CACHE:
        _CACHE["nc"] = _build_bass()
    nc = _CACHE["nc"]

    wrep, wshift, biasrow, ones = _consts_np(weight, bias)
    in_maps = []
    for b in range(B):
        in_maps.append(
            {
                "x": np.ascontiguousarray(x[b]),
                "state": np.ascontiguousarray(conv_state[b].T),
                "wrep": wrep,
                "wshift": wshift,
                "biasrow": biasrow,
                "ones": ones,
            }
        )

    kwargs = {}
    if TRACE:
        kwargs = dict(trace=True, trace_cores=[0])
    res = run_bass_kernel_spmd(nc, in_maps, core_ids=list(range(B)), **kwargs)
    LAST_RESULTS = res

    out = np.stack([res.results[b]["out"] for b in range(B)], axis=0)
    new_conv_state = np.ascontiguousarray(
        x[:, S - (K - 1) :, :].transpose(0, 2, 1), dtype=np.float32
    )
    return out, new_conv_state


# revision 4
# speedup vs baseline: 1.0018x; 1.0018x over previous
"""Depthwise causal conv1d (K=4) Trainium2 kernel.

Problem: x (B=8, S=4096, F=2048) f32, conv_state (B, F, 3), weight (F, 1, 4),
bias (F,).  out[b, s, f] = bias[f] + sum_d weight[f,0,d] * xpad[b, s+d, f]
where xpad = concat(conv_state[b].T, x[b]) along time.  Also returns
new_conv_state = xpad[:, -3:, :].T (computed on host -- it is a pure
input slice).

Sharding: batch across the 8 NeuronCores (one batch item per core), weights
replicated.  No cross-device communication.

Per-core layout: S on partitions, F on the free dim (natural DMA: each
partition line is a contiguous 8KB row of x).  Tiles of 124 output rows are
built from 127 input rows (3-row halo re-read).  Per tile:
  1. y_d = x_tile * wrep_d on the Vector engine only (tensor_tensor mult,
     f32 -> float32r out).  GpSimd is NOT used: it shares an SBUF read port
     with DVE and tensor_tensor on both engines concurrently is a net loss.
  2. PE accumulates out = sum_d Shift_d @ y_d in PSUM via float32r matmuls
     (0/1 shift weights are exact; data pays f32r rounding ~1e-4 rel).
     Bias is folded in: y_0 row 127 holds the bias row (8KB SBUF->SBUF DMA)
     and Shift_0 row 127 is all-ones.
  3. ACT (ScalarE) evacuates PSUM -> SBUF, HWDGE DMA stores.
"""

import numpy as np

B, S, F, K = 8, 4096, 2048, 4
P = 128
TILE_OUT = 124  # output rows per tile; 127 input rows + bias row at 127
N_TILES = (S + TILE_OUT - 1) // TILE_OUT  # 34 (last tile: 4 rows)
CHUNK = 512  # PSUM bank = 512 fp32

_CACHE = {}
LAST_RESULTS = None
TRACE = False


def _build_bass():
    import concourse.tile as tile
    import concourse.bacc as bacc
    from concourse import mybir

    f32 = mybir.dt.float32
    f32r = mybir.dt.float32r

    nc = bacc.Bacc("TRN2", target_bir_lowering=False, debug=False)

    x_dram = nc.dram_tensor("x", (S, F), f32, kind="ExternalInput")
    state_dram = nc.dram_tensor("state", (K - 1, F), f32, kind="ExternalInput")
    wrep_dram = nc.dram_tensor("wrep", (K, P, F), f32, kind="ExternalInput")
    wshift_dram = nc.dram_tensor("wshift", (K, P, P), f32, kind="ExternalInput")
    biasrow_dram = nc.dram_tensor("biasrow", (1, F), f32, kind="ExternalInput")
    ones_dram = nc.dram_tensor("ones", (1, P), f32, kind="ExternalInput")
    onesrow_dram = nc.dram_tensor("onesrow", (1, F), f32, kind="ExternalInput")
    out_dram = nc.dram_tensor("out", (S, F), f32, kind="ExternalOutput")

    NCH = F // CHUNK

    with tile.TileContext(nc) as tc:
        with (
            tc.tile_pool(name="consts", bufs=1) as consts,
            tc.tile_pool(name="xp", bufs=3) as xp,
            tc.tile_pool(name="yp", bufs=2) as yp,
            tc.tile_pool(name="op", bufs=3) as op,
            tc.tile_pool(name="psum", bufs=2, space="PSUM") as pp,
        ):
            wrep = []
            wshift = []
            for d in range(K):
                wr = consts.tile([P, F], f32, tag=f"wrep{d}")
                nc.sync.dma_start(wr[:], wrep_dram.ap()[d])
                wrep.append(wr)
                wsf = consts.tile([P, P], f32, tag=f"wshiftf{d}")
                nc.sync.dma_start(wsf[:], wshift_dram.ap()[d])
                ws = consts.tile([P, P], f32r, tag=f"wshift{d}")
                nc.vector.tensor_copy(ws[:], wsf[:])
                wshift.append(ws)
            biasrow_f = consts.tile([1, F], f32)
            nc.sync.dma_start(biasrow_f[:], biasrow_dram.ap())
            biasrow = consts.tile([1, F], f32r, tag="biasrow_r")
            nc.vector.tensor_copy(biasrow[:], biasrow_f[:])
            ones_f = consts.tile([1, P], f32)
            nc.sync.dma_start(ones_f[:], ones_dram.ap())
            ones = consts.tile([1, P], f32r, tag="ones_r")
            nc.vector.tensor_copy(ones[:], ones_f[:])

            for j in range(N_TILES):
                r0 = TILE_OUT * j
                n_out = min(TILE_OUT, S - r0)
                n_in = n_out + (K - 1)
                full = n_out == TILE_OUT

                xt = xp.tile([P, F], f32)
                if j == 0:
                    nc.sync.dma_start(xt[0 : K - 1, :], state_dram.ap())
                    nc.sync.dma_start(xt[K - 1 : n_in, :], x_dram.ap()[0:n_out, :])
                else:
                    nc.sync.dma_start(
                        xt[0:n_in, :], x_dram.ap()[r0 - (K - 1) : r0 + n_out, :]
                    )
                if full:
                    # ones row at partition 127; with wrep_0[127] = bias the
                    # y_0 multiply turns it into the bias row, which W_0's
                    # all-ones row 127 then adds to every output row.
                    nc.sync.dma_start(xt[127:128, :], onesrow_dram.ap())

                ys = []
                for d in range(K):
                    y = yp.tile([P, F], f32r, tag=f"y{d}")
                    rows = P if (full and d == 0) else n_in
                    nc.vector.tensor_mul(
                        y[0:rows, :], xt[0:rows, :], wrep[d][0:rows, :]
                    )
                    ys.append(y)

                acc = pp.tile([P, F], f32)
                if full:
                    # d-outer order: weight reused across the 4 chunks
                    for d in range(K):
                        rows = P if d == 0 else n_in
                        for c in range(NCH):
                            sl = slice(CHUNK * c, CHUNK * (c + 1))
                            nc.tensor.matmul(
                                acc[:, sl],
                                wshift[d][0:rows, :],
                                ys[d][0:rows, sl],
                                start=(d == 0),
                                stop=(d == K - 1),
                            )
                else:
                    # short last tile: plain shifts + explicit bias matmul
                    for d in range(K):
                        for c in range(NCH):
                            sl = slice(CHUNK * c, CHUNK * (c + 1))
                            nc.tensor.matmul(
                                acc[:, sl],
                                wshift[d][0:n_in, :],
                                ys[d][0:n_in, sl],
                                start=(d == 0),
                                stop=False,
                            )
                    for c in range(NCH):
                        sl = slice(CHUNK * c, CHUNK * (c + 1))
                        nc.tensor.matmul(
                            acc[:, sl], ones[:], biasrow[:, sl],
                            start=False, stop=True,
                        )

                ot = op.tile([TILE_OUT, F], f32)
                nc.scalar.copy(ot[0:n_out, :], acc[0:n_out, :])
                nc.sync.dma_start(out_dram.ap()[r0 : r0 + n_out, :], ot[0:n_out, :])

    nc.compile()
    return nc


def _consts_np(weight, bias):
    w = weight[:, 0, :].astype(np.float32)  # (F, K)
    wrep = np.ascontiguousarray(
        np.broadcast_to(w.T[:, None, :], (K, P, F)), dtype=np.float32
    )
    wshift = np.zeros((K, P, P), dtype=np.float32)
    for d in range(K):
        for m in range(P - d):
            wshift[d, m + d, m] = 1.0
    wshift[0, 127, :] = 1.0  # bias row: out[:, m] += y0[127] = bias
    wrep[0, 127, :] = bias  # y0[127] = ones * bias = bias
    biasrow = np.ascontiguousarray(bias[None, :], dtype=np.float32)
    ones = np.ones((1, P), dtype=np.float32)
    onesrow = np.ones((1, F), dtype=np.float32)
    return wrep, wshift, biasrow, ones, onesrow


def kernel(x, conv_state, weight, bias):
    global LAST_RESULTS
    from concourse.bass_utils import run_bass_kernel_spmd

    x = np.asarray(x, dtype=np.float32)
    conv_state = np.asarray(conv_state, dtype=np.float32)
    weight = np.asarray(weight, dtype=np.float32)
    bias = np.asarray(bias, dtype=np.float32)

    if "nc" not in _CACHE:
        _CACHE["nc"] = _build_bass()
    nc = _CACHE["nc"]

    wrep, wshift, biasrow, ones, onesrow = _consts_np(weight, bias)
    in_maps = []
    for b in range(B):
        in_maps.append(
            {
                "x": np.ascontiguousarray(x[b]),
                "state": np.ascontiguousarray(conv_state[b].T),
                "wrep": wrep,
                "wshift": wshift,
                "biasrow": biasrow,
                "ones": ones,
                "onesrow": onesrow,
            }
        )

    kwargs = {}
    if TRACE:
        kwargs = dict(trace=True, trace_cores=[0])
    res = run_bass_kernel_spmd(nc, in_maps, core_ids=list(range(B)), **kwargs)
    LAST_RESULTS = res

    out = np.stack([res.results[b]["out"] for b in range(B)], axis=0)
    new_conv_state = np.ascontiguousarray(
        x[:, S - (K - 1) :, :].transpose(0, 2, 1), dtype=np.float32
    )
    return out, new_conv_state


# revision 5
# speedup vs baseline: 1.0048x; 1.0029x over previous
"""Depthwise causal conv1d (K=4) Trainium2 kernel.

Problem: x (B=8, S=4096, F=2048) f32, conv_state (B, F, 3), weight (F, 1, 4),
bias (F,).  out[b, s, f] = bias[f] + sum_d weight[f,0,d] * xpad[b, s+d, f]
where xpad = concat(conv_state[b].T, x[b]) along time.  Also returns
new_conv_state = xpad[:, -3:, :].T (computed on host -- it is a pure
input slice).

Sharding: batch across the 8 NeuronCores (one batch item per core), weights
replicated.  No cross-device communication.

Per-core layout: S on partitions, F on the free dim (natural DMA: each
partition line is a contiguous 8KB row of x).  Tiles of 124 output rows are
built from 127 input rows (3-row halo re-read).  Per tile:
  1. y_d = x_tile * wrep_d on the Vector engine only (tensor_tensor mult,
     f32 -> float32r out).  GpSimd is NOT used: it shares an SBUF read port
     with DVE and tensor_tensor on both engines concurrently is a net loss.
  2. PE accumulates out = sum_d Shift_d @ y_d in PSUM via float32r matmuls
     (0/1 shift weights are exact; data pays f32r rounding ~1e-4 rel).
     Bias is folded in: y_0 row 127 holds the bias row (8KB SBUF->SBUF DMA)
     and Shift_0 row 127 is all-ones.
  3. ACT (ScalarE) evacuates PSUM -> SBUF, HWDGE DMA stores.
"""

import numpy as np

B, S, F, K = 8, 4096, 2048, 4
P = 128
TILE_OUT = 124  # output rows per tile; 127 input rows + bias row at 127
N_TILES = (S + TILE_OUT - 1) // TILE_OUT  # 34 (last tile: 4 rows)
CHUNK = 512  # PSUM bank = 512 fp32

_CACHE = {}
LAST_RESULTS = None
TRACE = False


def _build_bass():
    import concourse.tile as tile
    import concourse.bacc as bacc
    from concourse import mybir

    f32 = mybir.dt.float32
    f32r = mybir.dt.float32r

    nc = bacc.Bacc("TRN2", target_bir_lowering=False, debug=False)

    x_dram = nc.dram_tensor("x", (S, F), f32, kind="ExternalInput")
    state_dram = nc.dram_tensor("state", (K - 1, F), f32, kind="ExternalInput")
    wrep_dram = nc.dram_tensor("wrep", (K, P, F), f32, kind="ExternalInput")
    wshift_dram = nc.dram_tensor("wshift", (K, P, P), f32, kind="ExternalInput")
    biasrow_dram = nc.dram_tensor("biasrow", (1, F), f32, kind="ExternalInput")
    ones_dram = nc.dram_tensor("ones", (1, P), f32, kind="ExternalInput")
    onesrow_dram = nc.dram_tensor("onesrow", (1, F), f32, kind="ExternalInput")
    out_dram = nc.dram_tensor("out", (S, F), f32, kind="ExternalOutput")

    NCH = F // CHUNK

    with tile.TileContext(nc) as tc:
        with (
            tc.tile_pool(name="consts", bufs=1) as consts,
            tc.tile_pool(name="xp", bufs=3) as xp,
            tc.tile_pool(name="yp", bufs=2) as yp,
            tc.tile_pool(name="op", bufs=3) as op,
            tc.tile_pool(name="psum", bufs=2, space="PSUM") as pp,
        ):
            wrep = []
            wshift = []
            for d in range(K):
                wr = consts.tile([P, F], f32, tag=f"wrep{d}")
                nc.sync.dma_start(wr[:], wrep_dram.ap()[d])
                wrep.append(wr)
                wsf = consts.tile([P, P], f32, tag=f"wshiftf{d}")
                nc.sync.dma_start(wsf[:], wshift_dram.ap()[d])
                ws = consts.tile([P, P], f32r, tag=f"wshift{d}")
                nc.vector.tensor_copy(ws[:], wsf[:])
                wshift.append(ws)
            biasrow_f = consts.tile([1, F], f32)
            nc.sync.dma_start(biasrow_f[:], biasrow_dram.ap())
            biasrow = consts.tile([1, F], f32r, tag="biasrow_r")
            nc.vector.tensor_copy(biasrow[:], biasrow_f[:])
            ones_f = consts.tile([1, P], f32)
            nc.sync.dma_start(ones_f[:], ones_dram.ap())
            ones = consts.tile([1, P], f32r, tag="ones_r")
            nc.vector.tensor_copy(ones[:], ones_f[:])

            for j in range(N_TILES):
                r0 = TILE_OUT * j
                n_out = min(TILE_OUT, S - r0)
                n_in = n_out + (K - 1)
                full = n_out == TILE_OUT

                xt = xp.tile([P, F], f32)
                if j == 0:
                    nc.sync.dma_start(xt[0 : K - 1, :], state_dram.ap())
                    nc.sync.dma_start(xt[K - 1 : n_in, :], x_dram.ap()[0:n_out, :])
                else:
                    nc.sync.dma_start(
                        xt[0:n_in, :], x_dram.ap()[r0 - (K - 1) : r0 + n_out, :]
                    )
                ys = []
                for d in range(K):
                    y = yp.tile([P, F], f32r, tag=f"y{d}")
                    nc.vector.tensor_mul(
                        y[0:n_in, :], xt[0:n_in, :], wrep[d][0:n_in, :]
                    )
                    ys.append(y)

                acc = pp.tile([P, F], f32)
                # d-outer order: weight reused across the 4 chunks
                for d in range(K):
                    for c in range(NCH):
                        sl = slice(CHUNK * c, CHUNK * (c + 1))
                        nc.tensor.matmul(
                            acc[:, sl],
                            wshift[d][0:n_in, :],
                            ys[d][0:n_in, sl],
                            start=(d == 0),
                            stop=False,
                        )
                for c in range(NCH):
                    sl = slice(CHUNK * c, CHUNK * (c + 1))
                    nc.tensor.matmul(
                        acc[:, sl], ones[:], biasrow[:, sl],
                        start=False, stop=True,
                    )

                ot = op.tile([TILE_OUT, F], f32)
                nc.scalar.copy(ot[0:n_out, :], acc[0:n_out, :])
                nc.sync.dma_start(out_dram.ap()[r0 : r0 + n_out, :], ot[0:n_out, :])

    nc.compile()
    return nc


def _consts_np(weight, bias):
    w = weight[:, 0, :].astype(np.float32)  # (F, K)
    wrep = np.ascontiguousarray(
        np.broadcast_to(w.T[:, None, :], (K, P, F)), dtype=np.float32
    )
    wshift = np.zeros((K, P, P), dtype=np.float32)
    for d in range(K):
        for m in range(P - d):
            wshift[d, m + d, m] = 1.0
    biasrow = np.ascontiguousarray(bias[None, :], dtype=np.float32)
    ones = np.ones((1, P), dtype=np.float32)
    onesrow = np.ones((1, F), dtype=np.float32)
    return wrep, wshift, biasrow, ones, onesrow


def kernel(x, conv_state, weight, bias):
    global LAST_RESULTS
    from concourse.bass_utils import run_bass_kernel_spmd

    x = np.asarray(x, dtype=np.float32)
    conv_state = np.asarray(conv_state, dtype=np.float32)
    weight = np.asarray(weight, dtype=np.float32)
    bias = np.asarray(bias, dtype=np.float32)

    if "nc" not in _CACHE:
        _CACHE["nc"] = _build_bass()
    nc = _CACHE["nc"]

    wrep, wshift, biasrow, ones, onesrow = _consts_np(weight, bias)
    in_maps = []
    for b in range(B):
        in_maps.append(
            {
                "x": np.ascontiguousarray(x[b]),
                "state": np.ascontiguousarray(conv_state[b].T),
                "wrep": wrep,
                "wshift": wshift,
                "biasrow": biasrow,
                "ones": ones,
                "onesrow": onesrow,
            }
        )

    kwargs = {}
    if TRACE:
        kwargs = dict(trace=True, trace_cores=[0])
    res = run_bass_kernel_spmd(nc, in_maps, core_ids=list(range(B)), **kwargs)
    LAST_RESULTS = res

    out = np.stack([res.results[b]["out"] for b in range(B)], axis=0)
    new_conv_state = np.ascontiguousarray(
        x[:, S - (K - 1) :, :].transpose(0, 2, 1), dtype=np.float32
    )
    return out, new_conv_state


# revision 6
# speedup vs baseline: 4.1392x; 4.1196x over previous
"""Depthwise causal conv1d (K=4) Trainium2 kernel.

Problem: x (B=8, S=4096, F=2048) f32, conv_state (B, F, 3), weight (F, 1, 4),
bias (F,).  out[b, s, f] = bias[f] + sum_d weight[f,0,d] * xpad[b, s+d, f]
where xpad = concat(conv_state[b].T, x[b]) along time.  Also returns
new_conv_state = xpad[:, -3:, :].T (computed on host -- it is a pure
input slice).

Sharding: batch across the 8 NeuronCores (one batch item per core), weights
replicated.  No cross-device communication.

Per-core layout: S on partitions, F on the free dim (natural DMA: each
partition line is a contiguous 8KB row of x).  Tiles of 124 output rows are
built from 127 input rows (3-row halo re-read).  Per tile:
  1. y_d = x_tile * wrep_d on the Vector engine only (tensor_tensor mult,
     f32 -> float32r out).  GpSimd is NOT used: it shares an SBUF read port
     with DVE and tensor_tensor on both engines concurrently is a net loss.
  2. PE accumulates out = sum_d Shift_d @ y_d in PSUM via float32r matmuls
     (0/1 shift weights are exact; data pays f32r rounding ~1e-4 rel).
     Bias is folded in: y_0 row 127 holds the bias row (8KB SBUF->SBUF DMA)
     and Shift_0 row 127 is all-ones.
  3. ACT (ScalarE) evacuates PSUM -> SBUF, HWDGE DMA stores.
"""

import numpy as np

B, S, F, K = 8, 4096, 2048, 4
P = 128
TILE_OUT = 125  # output rows per tile; 128 input rows (3-row halo)
N_TILES = (S + TILE_OUT - 1) // TILE_OUT  # 34 (last tile: 4 rows)
CHUNK = 512  # PSUM bank = 512 fp32

_CACHE = {}
LAST_RESULTS = None
TRACE = False


def _build_bass():
    import concourse.tile as tile
    import concourse.bacc as bacc
    from concourse import mybir

    f32 = mybir.dt.float32
    f32r = mybir.dt.float32r

    nc = bacc.Bacc("TRN2", target_bir_lowering=False, debug=False)

    x_dram = nc.dram_tensor("x", (S, F), f32, kind="ExternalInput")
    state_dram = nc.dram_tensor("state", (K - 1, F), f32, kind="ExternalInput")
    wrep_dram = nc.dram_tensor("wrep", (K, P, F), f32, kind="ExternalInput")
    wshift_dram = nc.dram_tensor("wshift", (K, P, P), f32, kind="ExternalInput")
    biasrow_dram = nc.dram_tensor("biasrow", (1, F), f32, kind="ExternalInput")
    ones_dram = nc.dram_tensor("ones", (1, P), f32, kind="ExternalInput")
    onesrow_dram = nc.dram_tensor("onesrow", (1, F), f32, kind="ExternalInput")
    out_dram = nc.dram_tensor("out", (S, F), f32, kind="ExternalOutput")

    NCH = F // CHUNK

    with tile.TileContext(nc) as tc:
        with (
            tc.tile_pool(name="consts", bufs=1) as consts,
            tc.tile_pool(name="xp", bufs=3) as xp,
            tc.tile_pool(name="yp", bufs=2) as yp,
            tc.tile_pool(name="op", bufs=3) as op,
            tc.tile_pool(name="psum", bufs=2, space="PSUM") as pp,
        ):
            wrep = []
            wshift = []
            for d in range(K):
                wr = consts.tile([P, F], f32, tag=f"wrep{d}")
                nc.sync.dma_start(wr[:], wrep_dram.ap()[d])
                wrep.append(wr)
                wsf = consts.tile([P, P], f32, tag=f"wshiftf{d}")
                nc.sync.dma_start(wsf[:], wshift_dram.ap()[d])
                ws = consts.tile([P, P], f32r, tag=f"wshift{d}")
                nc.vector.tensor_copy(ws[:], wsf[:])
                wshift.append(ws)
            biasrow_f = consts.tile([1, F], f32)
            nc.sync.dma_start(biasrow_f[:], biasrow_dram.ap())
            biasrow = consts.tile([1, F], f32r, tag="biasrow_r")
            nc.vector.tensor_copy(biasrow[:], biasrow_f[:])
            ones_f = consts.tile([1, P], f32)
            nc.sync.dma_start(ones_f[:], ones_dram.ap())
            ones = consts.tile([1, P], f32r, tag="ones_r")
            nc.vector.tensor_copy(ones[:], ones_f[:])

            for j in range(N_TILES):
                r0 = TILE_OUT * j
                n_out = min(TILE_OUT, S - r0)
                n_in = n_out + (K - 1)
                full = n_out == TILE_OUT

                xt = xp.tile([P, F], f32)
                if j == 0:
                    nc.sync.dma_start(xt[0 : K - 1, :], state_dram.ap())
                    nc.sync.dma_start(xt[K - 1 : n_in, :], x_dram.ap()[0:n_out, :])
                else:
                    nc.sync.dma_start(
                        xt[0:n_in, :], x_dram.ap()[r0 - (K - 1) : r0 + n_out, :]
                    )
                ys = []
                for d in range(K):
                    y = yp.tile([P, F], f32r, tag=f"y{d}")
                    nc.vector.tensor_mul(
                        y[0:n_in, :], xt[0:n_in, :], wrep[d][0:n_in, :]
                    )
                    ys.append(y)

                acc = pp.tile([P, F], f32)
                # d-outer order: weight reused across the 4 chunks
                for d in range(K):
                    for c in range(NCH):
                        sl = slice(CHUNK * c, CHUNK * (c + 1))
                        nc.tensor.matmul(
                            acc[:, sl],
                            wshift[d][0:n_in, :],
                            ys[d][0:n_in, sl],
                            start=(d == 0),
                            stop=False,
                        )
                for c in range(NCH):
                    sl = slice(CHUNK * c, CHUNK * (c + 1))
                    nc.tensor.matmul(
                        acc[:, sl], ones[:], biasrow[:, sl],
                        start=False, stop=True,
                    )

                ot = op.tile([TILE_OUT, F], f32)
                nc.scalar.copy(ot[0:n_out, :], acc[0:n_out, :])
                nc.sync.dma_start(out_dram.ap()[r0 : r0 + n_out, :], ot[0:n_out, :])

    nc.compile()
    return nc


def _consts_np(weight, bias):
    w = weight[:, 0, :].astype(np.float32)  # (F, K)
    wrep = np.ascontiguousarray(
        np.broadcast_to(w.T[:, None, :], (K, P, F)), dtype=np.float32
    )
    wshift = np.zeros((K, P, P), dtype=np.float32)
    for d in range(K):
        for m in range(P - d):
            wshift[d, m + d, m] = 1.0
    biasrow = np.ascontiguousarray(bias[None, :], dtype=np.float32)
    ones = np.ones((1, P), dtype=np.float32)
    onesrow = np.ones((1, F), dtype=np.float32)
    return wrep, wshift, biasrow, ones, onesrow


def kernel(x, conv_state, weight, bias):
    global LAST_RESULTS
    from concourse.bass_utils import run_bass_kernel_spmd

    x = np.asarray(x, dtype=np.float32)
    conv_state = np.asarray(conv_state, dtype=np.float32)
    weight = np.asarray(weight, dtype=np.float32)
    bias = np.asarray(bias, dtype=np.float32)

    if "nc" not in _CACHE:
        _CACHE["nc"] = _build_bass()
    nc = _CACHE["nc"]

    wrep, wshift, biasrow, ones, onesrow = _consts_np(weight, bias)
    in_maps = []
    for b in range(B):
        in_maps.append(
            {
                "x": np.ascontiguousarray(x[b]),
                "state": np.ascontiguousarray(conv_state[b].T),
                "wrep": wrep,
                "wshift": wshift,
                "biasrow": biasrow,
                "ones": ones,
                "onesrow": onesrow,
            }
        )

    kwargs = {}
    if TRACE:
        kwargs = dict(trace=True, trace_cores=[0])
    res = run_bass_kernel_spmd(nc, in_maps, core_ids=list(range(B)), **kwargs)
    LAST_RESULTS = res

    out = np.stack([res.results[b]["out"] for b in range(B)], axis=0)
    new_conv_state = np.ascontiguousarray(
        x[:, S - (K - 1) :, :].transpose(0, 2, 1), dtype=np.float32
    )
    return out, new_conv_state


# revision 7
# speedup vs baseline: 4.1650x; 1.0062x over previous
"""Depthwise causal conv1d (K=4) Trainium2 kernel.

Problem: x (B=8, S=4096, F=2048) f32, conv_state (B, F, 3), weight (F, 1, 4),
bias (F,).  out[b, s, f] = bias[f] + sum_d weight[f,0,d] * xpad[b, s+d, f]
where xpad = concat(conv_state[b].T, x[b]) along time.  Also returns
new_conv_state = xpad[:, -3:, :].T (computed on host -- it is a pure
input slice).

Sharding: batch across the 8 NeuronCores (one batch item per core), weights
replicated.  No cross-device communication.

Per-core layout: S on partitions, F on the free dim (natural DMA: each
partition line is a contiguous 8KB row of x).  Tiles of 124 output rows are
built from 127 input rows (3-row halo re-read).  Per tile:
  1. y_d = x_tile * wrep_d on the Vector engine only (tensor_tensor mult,
     f32 -> float32r out).  GpSimd is NOT used: it shares an SBUF read port
     with DVE and tensor_tensor on both engines concurrently is a net loss.
  2. PE accumulates out = sum_d Shift_d @ y_d in PSUM via float32r matmuls
     (0/1 shift weights are exact; data pays f32r rounding ~1e-4 rel).
     Bias is folded in: y_0 row 127 holds the bias row (8KB SBUF->SBUF DMA)
     and Shift_0 row 127 is all-ones.
  3. ACT (ScalarE) evacuates PSUM -> SBUF, HWDGE DMA stores.
"""

import numpy as np

B, S, F, K = 8, 4096, 2048, 4
P = 128
TILE_OUT = 125  # output rows per tile; 128 input rows (3-row halo)
N_TILES = (S + TILE_OUT - 1) // TILE_OUT  # 34 (last tile: 4 rows)
CHUNK = 512  # PSUM bank = 512 fp32

_CACHE = {}
LAST_RESULTS = None
TRACE = False


def _build_bass():
    import concourse.tile as tile
    import concourse.bacc as bacc
    from concourse import mybir

    f32 = mybir.dt.float32
    f32r = mybir.dt.float32r
    bf16 = mybir.dt.bfloat16

    nc = bacc.Bacc("TRN2", target_bir_lowering=False, debug=False)

    x_dram = nc.dram_tensor("x", (S, F), f32, kind="ExternalInput")
    state_dram = nc.dram_tensor("state", (K - 1, F), f32, kind="ExternalInput")
    wrep_dram = nc.dram_tensor("wrep", (K, P, F), f32, kind="ExternalInput")
    wshift_dram = nc.dram_tensor("wshift", (K, P, P), f32, kind="ExternalInput")
    biasrow_dram = nc.dram_tensor("biasrow", (1, F), f32, kind="ExternalInput")
    ones_dram = nc.dram_tensor("ones", (1, P), f32, kind="ExternalInput")
    onesrow_dram = nc.dram_tensor("onesrow", (1, F), f32, kind="ExternalInput")
    out_dram = nc.dram_tensor("out", (S, F), f32, kind="ExternalOutput")

    NCH = F // CHUNK

    with tile.TileContext(nc) as tc:
        with (
            tc.tile_pool(name="consts", bufs=1) as consts,
            tc.tile_pool(name="xp", bufs=3) as xp,
            tc.tile_pool(name="yp", bufs=2) as yp,
            tc.tile_pool(name="op", bufs=3) as op,
            tc.tile_pool(name="psum", bufs=2, space="PSUM") as pp,
        ):
            wrep = []
            wshift = []
            for d in range(K):
                wr = consts.tile([P, F], f32, tag=f"wrep{d}")
                nc.sync.dma_start(wr[:], wrep_dram.ap()[d])
                wrep.append(wr)
                wsf = consts.tile([P, P], f32, tag=f"wshiftf{d}")
                nc.sync.dma_start(wsf[:], wshift_dram.ap()[d])
                ws = consts.tile([P, P], f32r, tag=f"wshift{d}")
                nc.vector.tensor_copy(ws[:], wsf[:])
                wshift.append(ws)
            biasrow_f = consts.tile([1, F], f32)
            nc.sync.dma_start(biasrow_f[:], biasrow_dram.ap())
            biasrow = consts.tile([1, F], f32r, tag="biasrow_r")
            nc.vector.tensor_copy(biasrow[:], biasrow_f[:])
            ones_f = consts.tile([1, P], f32)
            nc.sync.dma_start(ones_f[:], ones_dram.ap())
            ones = consts.tile([1, P], f32r, tag="ones_r")
            nc.vector.tensor_copy(ones[:], ones_f[:])
            heater_w = consts.tile([P, P], bf16, tag="heater_w")
            nc.vector.tensor_copy(heater_w[:], wsf[:])

            for j in range(N_TILES):
                r0 = TILE_OUT * j
                n_out = min(TILE_OUT, S - r0)
                n_in = n_out + (K - 1)
                full = n_out == TILE_OUT

                xt = xp.tile([P, F], f32)
                if j == 0:
                    nc.sync.dma_start(xt[0 : K - 1, :], state_dram.ap())
                    nc.sync.dma_start(xt[K - 1 : n_in, :], x_dram.ap()[0:n_out, :])
                else:
                    nc.sync.dma_start(
                        xt[0:n_in, :], x_dram.ap()[r0 - (K - 1) : r0 + n_out, :]
                    )
                ys = []
                for d in range(K):
                    y = yp.tile([P, F], f32r, tag=f"y{d}")
                    nc.vector.tensor_mul(
                        y[0:n_in, :], xt[0:n_in, :], wrep[d][0:n_in, :]
                    )
                    ys.append(y)

                acc = pp.tile([P, F], f32)
                # PE heaters: tiny bf16 matmuls keyed to each y_d so the PE
                # wakes during the DVE window and HAM stays at full clock.
                # They write a corner of acc that the real start=True matmul
                # clears right after.
                for d in (1, 2, 3):
                    hb = ys[d][:, 0:32].bitcast(bf16)
                    nc.tensor.matmul(
                        acc[0:64, 0:64],
                        heater_w[:, 0:64],
                        hb[:, 0:64],
                        start=True,
                        stop=True,
                        skip_group_check=True,
                    )
                # d-outer order: weight reused across the 4 chunks
                for d in range(K):
                    for c in range(NCH):
                        sl = slice(CHUNK * c, CHUNK * (c + 1))
                        nc.tensor.matmul(
                            acc[:, sl],
                            wshift[d][0:n_in, :],
                            ys[d][0:n_in, sl],
                            start=(d == 0),
                            stop=False,
                        )
                for c in range(NCH):
                    sl = slice(CHUNK * c, CHUNK * (c + 1))
                    nc.tensor.matmul(
                        acc[:, sl], ones[:], biasrow[:, sl],
                        start=False, stop=True,
                    )

                ot = op.tile([TILE_OUT, F], f32)
                nc.scalar.copy(ot[0:n_out, :], acc[0:n_out, :])
                nc.sync.dma_start(out_dram.ap()[r0 : r0 + n_out, :], ot[0:n_out, :])

    nc.compile()
    return nc


def _consts_np(weight, bias):
    w = weight[:, 0, :].astype(np.float32)  # (F, K)
    wrep = np.ascontiguousarray(
        np.broadcast_to(w.T[:, None, :], (K, P, F)), dtype=np.float32
    )
    wshift = np.zeros((K, P, P), dtype=np.float32)
    for d in range(K):
        for m in range(P - d):
            wshift[d, m + d, m] = 1.0
    biasrow = np.ascontiguousarray(bias[None, :], dtype=np.float32)
    ones = np.ones((1, P), dtype=np.float32)
    onesrow = np.ones((1, F), dtype=np.float32)
    return wrep, wshift, biasrow, ones, onesrow


def kernel(x, conv_state, weight, bias):
    global LAST_RESULTS
    from concourse.bass_utils import run_bass_kernel_spmd

    x = np.asarray(x, dtype=np.float32)
    conv_state = np.asarray(conv_state, dtype=np.float32)
    weight = np.asarray(weight, dtype=np.float32)
    bias = np.asarray(bias, dtype=np.float32)

    if "nc" not in _CACHE:
        _CACHE["nc"] = _build_bass()
    nc = _CACHE["nc"]

    wrep, wshift, biasrow, ones, onesrow = _consts_np(weight, bias)
    in_maps = []
    for b in range(B):
        in_maps.append(
            {
                "x": np.ascontiguousarray(x[b]),
                "state": np.ascontiguousarray(conv_state[b].T),
                "wrep": wrep,
                "wshift": wshift,
                "biasrow": biasrow,
                "ones": ones,
                "onesrow": onesrow,
            }
        )

    kwargs = {}
    if TRACE:
        kwargs = dict(trace=True, trace_cores=[0])
    res = run_bass_kernel_spmd(nc, in_maps, core_ids=list(range(B)), **kwargs)
    LAST_RESULTS = res

    out = np.stack([res.results[b]["out"] for b in range(B)], axis=0)
    new_conv_state = np.ascontiguousarray(
        x[:, S - (K - 1) :, :].transpose(0, 2, 1), dtype=np.float32
    )
    return out, new_conv_state
